# revision 4
# baseline (speedup 1.0000x reference)
"""HQQ int4 weight-only quantized linear for TRN2, 8-core tensor-parallel.

out[M, N] = x[M, K] @ dequant(W_q[N, K]).T
  dequant: w[n, k] = (q[n, k] - 8) * scales[n, k//128] + zeros[n, k//128]

Sharding: column-parallel over N (out_features) across 8 NeuronCores;
x replicated; outputs concatenated on host. No collectives.

Host prep is value-preserving layout work only: the int4 codes are shifted
(-8, exact) and widened to bf16, and tensors are transposed into the
K-major layouts the TensorEngine needs. The dequant math (*scale, +zero)
and the full matmul run on device.
"""

import os
import sys

import numpy as np
import ml_dtypes

M = 4096
K = 4096
N = 11008
GROUP = 128
N_CORES = 8
N_SHARD = N // N_CORES  # 1376
NG = K // GROUP  # 32 quant groups == 32 k-tiles of 128
BF16 = ml_dtypes.bfloat16


def _install_axon_hooks_shim():
    """antenv.axon_hooks is missing from this image; run_bass_kernel_spmd
    imports it when tracing is requested (e.g. BASS_TRACE=1). Provide the
    same ctypes-based hook trn_boot would have registered."""
    import types

    try:
        import antenv.axon_hooks  # noqa: F401

        return
    except ImportError:
        pass
    try:
        import antenv
        from trn_agent_boot.trn_boot import _ntff_profile_via_ctypes

        hook = _ntff_profile_via_ctypes("/opt/axon/libaxon_pjrt.so")
        mod = types.ModuleType("antenv.axon_hooks")
        mod._hook = hook
        mod.get_axon_ntff_profile_hook = lambda: mod._hook

        def _set(h):
            mod._hook = h

        mod.set_axon_ntff_profile_hook = _set
        sys.modules["antenv.axon_hooks"] = mod
        antenv.axon_hooks = mod
    except Exception:
        pass


def build_bass(m=M, k=K, n_shard=N_SHARD, ng=NG, compile=True):
    """Build the per-core Bass module. Parameterized so a scaled-down
    config can be checked in CoreSim."""
    import concourse.mybir as mybir
    import concourse.tile as tile
    from concourse import bacc

    P = 128
    assert k == ng * GROUP and m % 512 == 0
    f32 = mybir.dt.float32
    bf16 = mybir.dt.bfloat16

    nc = bacc.Bacc("TRN2", target_bir_lowering=False, debug=False)
    xT = nc.dram_tensor("xT", [k, m], bf16, kind="ExternalInput")
    w8 = nc.dram_tensor("w8", [k, n_shard], bf16, kind="ExternalInput")
    sT = nc.dram_tensor("sT", [ng, n_shard], bf16, kind="ExternalInput")
    zT = nc.dram_tensor("zT", [ng, n_shard], bf16, kind="ExternalInput")
    out = nc.dram_tensor("out", [m, n_shard], bf16, kind="ExternalOutput")

    # n-tiles of <=512 (PSUM bank free-dim limit)
    n_tiles = []
    st = 0
    while st < n_shard:
        nf = min(512, n_shard - st)
        n_tiles.append((st, nf))
        st += nf

    M_PANEL = 512
    n_panels = m // M_PANEL

    with tile.TileContext(nc) as tc:
        with (
            tc.tile_pool(name="wdeq", bufs=ng) as wdeq_pool,
            tc.tile_pool(name="bc", bufs=4) as bc_pool,
            tc.tile_pool(name="xp", bufs=2) as xp_pool,
            tc.tile_pool(name="osb", bufs=3) as osb_pool,
            tc.tile_pool(name="psum", bufs=6, space="PSUM") as psum_pool,
        ):
            # ---- prefetch first x panel before dequant traffic ----
            xT3 = xT.rearrange("(ko ki) mm -> ki ko mm", ki=P)
            xp_tiles = {}
            xp_tiles[0] = xp_pool.tile([P, ng, M_PANEL], bf16, tag="xp", name="xp0")
            nc.sync.dma_start(xp_tiles[0][:], xT3[:, :, 0:M_PANEL])

            # ---- dequant: 32 k-group tiles, resident in SBUF, in place ----
            wdeq_tiles = []
            for g in range(ng):
                wd = wdeq_pool.tile([P, n_shard], bf16, tag="wdeq")
                nc.sync.dma_start(wd[:], w8[g * P : (g + 1) * P, :])
                s_bc = bc_pool.tile([P, n_shard], bf16, tag="sbc")
                nc.scalar.dma_start(
                    s_bc[:], sT[g : g + 1, :].to_broadcast((P, n_shard))
                )
                z_bc = bc_pool.tile([P, n_shard], bf16, tag="zbc")
                nc.scalar.dma_start(
                    z_bc[:], zT[g : g + 1, :].to_broadcast((P, n_shard))
                )
                # split dequant math across DVE (2/3) and GpSimd (1/3)
                eng = nc.gpsimd if g % 3 == 2 else nc.vector
                eng.tensor_mul(wd[:], wd[:], s_bc[:])
                eng.tensor_add(wd[:], wd[:], z_bc[:])
                wdeq_tiles.append(wd)

            # ---- matmul: out[m_tile, n_tile] = sum_k xT[k,m].T @ wdeq[k,n] ----
            for mp in range(n_panels):
                if mp not in xp_tiles:
                    xp_tiles[mp] = xp_pool.tile([P, ng, M_PANEL], bf16, tag="xp", name=f"xp{mp}")
                    nc.sync.dma_start(
                        xp_tiles[mp][:],
                        xT3[:, :, mp * M_PANEL : (mp + 1) * M_PANEL],
                    )
                xp = xp_tiles[mp]
                for ms in range(M_PANEL // P):
                    psums = []
                    for st, nf in n_tiles:
                        ps = psum_pool.tile([P, 512], f32, tag="ps")
                        psums.append(ps[:, :nf])
                    for g in range(ng):
                        lhsT = xp[:, g, ms * P : (ms + 1) * P]
                        for j, (st, nf) in enumerate(n_tiles):
                            nc.tensor.matmul(
                                psums[j],
                                lhsT,
                                wdeq_tiles[g][:, st : st + nf],
                                start=(g == 0),
                                stop=(g == ng - 1),
                            )
                    osb = osb_pool.tile([P, n_shard], bf16, tag="osb")
                    for j, (st, nf) in enumerate(n_tiles):
                        nc.any.tensor_copy(osb[:, st : st + nf], psums[j])
                    m0 = mp * M_PANEL + ms * P
                    nc.sync.dma_start(out[m0 : m0 + P, :], osb[:])

    if compile:
        nc.compile()
    return nc


_NC_CACHE = {}


def kernel(x, W_q, scales, zeros):
    _install_axon_hooks_shim()
    from concourse.bass_utils import run_bass_kernel_spmd

    x = np.asarray(x)
    W_q = np.asarray(W_q)
    scales = np.asarray(scales)
    zeros = np.asarray(zeros)

    # Host layout prep (value-preserving).
    xT = np.ascontiguousarray(x.astype(BF16, copy=False).T)  # [K, M]
    w8_full = np.ascontiguousarray(
        (W_q.astype(np.float32) - 8.0).astype(BF16).T
    )  # [K, N], values in [-8, 7] (exact)
    sT_full = np.ascontiguousarray(scales.astype(BF16, copy=False).T)  # [NG, N]
    zT_full = np.ascontiguousarray(zeros.astype(BF16, copy=False).T)  # [NG, N]

    if "nc" not in _NC_CACHE:
        _NC_CACHE["nc"] = build_bass()
    nc = _NC_CACHE["nc"]

    in_maps = []
    for c in range(N_CORES):
        lo, hi = c * N_SHARD, (c + 1) * N_SHARD
        in_maps.append(
            {
                "xT": xT,
                "w8": np.ascontiguousarray(w8_full[:, lo:hi]),
                "sT": np.ascontiguousarray(sT_full[:, lo:hi]),
                "zT": np.ascontiguousarray(zT_full[:, lo:hi]),
            }
        )

    global _LAST_IN_MAPS
    _LAST_IN_MAPS = in_maps
    res = run_bass_kernel_spmd(nc, in_maps, list(range(N_CORES)))
    out = np.concatenate([res.results[c]["out"] for c in range(N_CORES)], axis=1)
    return out.astype(BF16, copy=False)


_LAST_IN_MAPS = None
